# revision 8
# baseline (speedup 1.0000x reference)
"""DCRNN (K=1, H0=0) fused kernel for 8 Trainium2 NeuronCores — fp8 v3.

Math (derived from the reference with H0 = 0):
    R is dead (multiplied by H0=0); XH == XHR == [x, 0].
    Az = (Wz[0] + Wz[1])[:F]           # [256, 32]
    Ah = (Wh[0] + Wh[1])[:F]           # [256, 32]
    Zc = sigmoid(-(x @ Az + bz))       # == 1 - Z, strictly positive
    T  = tanh(x @ Ah + bh)
    y  = (relu(Zc * T) @ Wl) + bl      # relu(Zc*T) == Zc>0 ? relu-placement exact

v3 strategy (per core, data-parallel over nodes):
  * x quantized host-side to fp8 e3m4 (xq = e3m4(c*x)); sign/scale folded
    into bf16 weights acat' = [-Az/c | Ah/c].  HBM traffic halves vs bf16.
  * Whole 6.4MB shard preloaded to SBUF with 6 large DMAs on one HWDGE queue.
  * Node-stationary matmuls: lhsT = fp8 x-chunk [128,128] (FWL 4x weight
    load), rhs = bf16 acat chunk [128,64], psum groups of 3 superblocks.
  * ScalarE: Sigmoid on z-cols, Tanh on t-cols (one table set), strided
    psum reads -> CONTIGUOUS sgz/sgt tiles so DVE runs in 2x/4x modes.
  * VectorE: u = sgt*sgz (2x), r = relu(u) (4x), g = r*Wl (2x), tree adds
    t1/t2 (2x).  GpSimdE (otherwise idle) finishes the tree: t3/t4/y.
  * y is PE-transposed in three flushes and stored via the scalar queue.
  * bz/bh are zero here; a generic variant adds them with a rank-1 matmul.
"""

import sys

import numpy as np

sys.path.insert(0, "/opt/trn_rl_repo")

import ml_dtypes

N = 200000
F = 256
HID = 32
NCORES = 8
PER = 25088            # padded nodes per core
NPAD = PER * NCORES    # 200704
SUPER = 1024           # nodes per superblock (8 subtiles of 128)
NSUPER = 25            # 24 full superblocks + 1 half (512 nodes)
YCOLS = PER // 128     # 196

XGROUPS = [2, 4, 6, 6, 6, 1]                    # x DMA groups (superblocks)
ACTG = [tuple(range(4 * i, 4 * i + 4)) for i in range(6)] + [(24,)]

BF16 = ml_dtypes.bfloat16
E3M4 = ml_dtypes.float8_e3m4
E3MAX = 15.5

_PROGS = {}


def _build_program(with_bias=False):
    import concourse.tile as tile
    from concourse import bacc, mybir

    BF = mybir.dt.bfloat16
    F8 = mybir.dt.float8e3
    F32 = mybir.dt.float32
    AF = mybir.ActivationFunctionType
    OP = mybir.AluOpType

    nc = bacc.Bacc("TRN2", target_bir_lowering=False, debug=False,
                   num_devices=NCORES)

    # host layout: [128, 50176] fp8; col = (b*2048 + c*1024 + j) for b<24,
    # tail b=24: 49152 + c*512 + j
    x_d = nc.dram_tensor("x", [128, 2 * PER], F8, kind="ExternalInput").ap()
    acat_d = nc.dram_tensor("acat", [2, 128, 64], BF, kind="ExternalInput").ap()
    wl_d = nc.dram_tensor("wlfull", [128, 1024], BF, kind="ExternalInput").ap()
    id_d = nc.dram_tensor("ident", [128, 128], F32, kind="ExternalInput").ap()
    bias_d = nc.dram_tensor("biascat", [1, 512], BF, kind="ExternalInput").ap()
    ones_d = nc.dram_tensor("ones", [1, 128], BF, kind="ExternalInput").ap()
    y_d = nc.dram_tensor("y", [YCOLS, 128], F32, kind="ExternalOutput").ap()

    with tile.TileContext(nc) as tc:
        with tc.tile_pool(name="const", bufs=1) as cp, \
             tc.tile_pool(name="xs", bufs=len(XGROUPS)) as xp, \
             tc.tile_pool(name="sg", bufs=4) as gp, \
             tc.tile_pool(name="dv", bufs=8) as vp, \
             tc.tile_pool(name="ps", bufs=2, space="PSUM") as pp:

            acat0 = cp.tile([128, 64], BF)
            acat1 = cp.tile([128, 64], BF)
            wlfull = cp.tile([128, 1024], BF)
            ident = cp.tile([128, 128], F32)
            ysb = cp.tile([128, YCOLS], F32)
            nc.scalar.dma_start(out=acat0[:], in_=acat_d[0])
            nc.scalar.dma_start(out=acat1[:], in_=acat_d[1])
            nc.scalar.dma_start(out=wlfull[:], in_=wl_d[:])
            nc.scalar.dma_start(out=ident[:], in_=id_d[:])
            if with_bias:
                biascat = cp.tile([1, 512], BF)
                ones = cp.tile([1, 128], BF)
                nc.scalar.dma_start(out=biascat[:], in_=bias_d[:])
                nc.scalar.dma_start(out=ones[:], in_=ones_d[:])

            # ---- preload the whole x shard (large DMAs, one HWDGE queue)
            xtiles = []      # (tile, first_super, n_supers)
            b0 = 0
            for ng in XGROUPS:
                xt = xp.tile([128, 12288], F8, tag="xt")
                c0 = b0 * 2048
                w = sum(2048 if (b0 + i) < NSUPER - 1 else 1024
                        for i in range(ng))
                nc.sync.dma_start(out=xt[:, :w], in_=x_d[:, c0:c0 + w])
                xtiles.append((xt, b0, ng))
                b0 += ng

            def lhs(b, c, s):
                """stationary fp8 x chunk [128, 128] for (superblock b,
                feature-chunk c, subtile s)."""
                for xt, g0, ng in xtiles:
                    if g0 <= b < g0 + ng:
                        nn = 1024 if b < NSUPER - 1 else 512
                        off = (b - g0) * 2048 + c * nn + s * 128
                        return xt[:, off:off + 128]
                raise AssertionError(b)

            # ---- main loop over ACT groups (3 superblocks each)
            ydone = 0
            yflush = 0
            for gi, supers in enumerate(ACTG):
                nsubg = sum((8 if b < NSUPER - 1 else 4) for b in supers)
                ncols = nsubg * 64
                pt = pp.tile([128, 2048], F32, tag="pt")
                col = 0
                for b in supers:
                    nsub = 8 if b < NSUPER - 1 else 4
                    for s in range(nsub):
                        out_sl = pt[:, col:col + 64]
                        if with_bias:
                            nc.tensor.matmul(out_sl, ones[:],
                                             biascat[:, :64],
                                             start=True, stop=False)
                        nc.tensor.matmul(out_sl, lhs(b, 0, s), acat0[:],
                                         start=not with_bias, stop=False)
                        nc.tensor.matmul(out_sl, lhs(b, 1, s), acat1[:],
                                         start=False, stop=True)
                        col += 64

                # split gates: sigmoid(z-cols) / tanh(t-cols), strided psum
                # reads, contiguous SBUF writes
                nz = nsubg * 32
                pt3 = pt[:, :ncols].rearrange("p (s h) -> p s h", h=64)
                sgz = gp.tile([128, 1024], BF, tag="sgz")
                sgt = gp.tile([128, 1024], BF, tag="sgt")
                sgz3 = sgz[:, :nz].rearrange("p (s h) -> p s h", h=32)
                sgt3 = sgt[:, :nz].rearrange("p (s h) -> p s h", h=32)
                nc.scalar.activation(sgz3, pt3[:, :, 0:32], AF.Sigmoid)
                nc.scalar.activation(sgt3, pt3[:, :, 32:64], AF.Tanh)

                # DVE: r = relu(sgt) (4x), u = r*sgz (2x), g = u*Wl (2x),
                # then one 1x reduce over HID
                r = vp.tile([128, 1024], BF, tag="r")
                nc.vector.tensor_scalar_max(r[:, :nz], sgt[:, :nz], 0.0)
                u = vp.tile([128, 1024], BF, tag="u")
                nc.vector.tensor_mul(u[:, :nz], r[:, :nz], sgz[:, :nz])
                g = vp.tile([128, 1024], BF, tag="g")
                nc.vector.tensor_mul(g[:, :nz], u[:, :nz], wlfull[:, :nz])
                g3 = g[:, :nz].rearrange("p (s h) -> p s h", h=32)
                yc0 = supers[0] * 8
                nc.vector.tensor_reduce(ysb[:, yc0:yc0 + nsubg], g3,
                                        axis=mybir.AxisListType.X, op=OP.add)
                ydone = yc0 + nsubg

                # flush finished slices of ysb to shorten the tail
                while yflush < len(_FLUSH) and ydone >= _FLUSH[yflush][1]:
                    h0, h1 = _FLUSH[yflush]
                    hw = h1 - h0
                    ytp = pp.tile([128, 2048], F32, tag="pt")
                    nc.tensor.transpose(ytp[:hw, :128], ysb[:, h0:h1],
                                        ident[:])
                    yts = vp.tile([128, 128], F32, tag="yts")
                    nc.vector.tensor_copy(yts[:hw, :], ytp[:hw, :128])
                    nc.sync.dma_start(out=y_d[h0:h1, :], in_=yts[:hw, :])
                    yflush += 1

    nc.compile()
    return nc


_FLUSH = [(0, 96), (96, 192), (192, 196)]


def _get_program(with_bias=False):
    if with_bias not in _PROGS:
        _PROGS[with_bias] = _build_program(with_bias)
    return _PROGS[with_bias]


def _host_inputs(x, Wz, bz, Wh, bh, Wl):
    x = np.asarray(x)
    Az = (np.asarray(Wz[0]) + np.asarray(Wz[1]))[:F]
    Ah = (np.asarray(Wh[0]) + np.asarray(Wh[1]))[:F]

    c = E3MAX / max(float(np.abs(x).max()), 1e-30)
    Acat = np.concatenate([-Az / c, Ah / c], axis=1)          # [256, 64]
    acat = np.stack([Acat[:128], Acat[128:]]).astype(BF16)    # [2, 128, 64]
    wlfull = np.tile(np.asarray(Wl).reshape(1, HID),
                     (128, 32)).astype(BF16)                  # [128, 1024]
    ident = np.eye(128, dtype=np.float32)
    biascat = np.concatenate([-np.asarray(bz), np.asarray(bh)])
    biascat8 = np.tile(biascat, 8)[None, :].astype(BF16)      # [1, 512]
    ones = np.ones((1, 128), BF16)

    # quantize + per-core transpose to [128, (b, c, j)] layout
    xq = np.clip(x * c, -E3MAX, E3MAX).astype(E3M4)
    xb = np.zeros((NPAD, F), dtype=E3M4)
    xb[:N] = xq
    shards = xb.reshape(NCORES, PER, F)
    nfull = (NSUPER - 1) * SUPER                              # 24576
    main = shards[:, :nfull].reshape(NCORES, NSUPER - 1, SUPER, 2, 128)
    main = main.transpose(0, 4, 1, 3, 2).reshape(NCORES, 128, -1)
    tail = shards[:, nfull:].reshape(NCORES, 1, PER - nfull, 2, 128)
    tail = tail.transpose(0, 4, 1, 3, 2).reshape(NCORES, 128, -1)
    xhost = np.concatenate([main, tail], axis=2)              # [NC, 128, 2*PER]
    return xhost, acat, wlfull, ident, biascat8, ones


def kernel(x, edge_index, Wz, bz, Wr, br, Wh, bh, Wl, bl):
    from concourse.bass_utils import run_bass_kernel_spmd

    xhost, acat, wlfull, ident, biascat8, ones = _host_inputs(
        x, Wz, bz, Wh, bh, Wl)
    with_bias = bool(np.any(np.asarray(bz)) or np.any(np.asarray(bh)))

    nc = _get_program(with_bias)
    in_maps = [{
        "x": np.ascontiguousarray(xhost[i]),
        "acat": acat,
        "wlfull": wlfull,
        "ident": ident,
        "biascat": biascat8,
        "ones": ones,
    } for i in range(NCORES)]

    res = run_bass_kernel_spmd(nc, in_maps, core_ids=list(range(NCORES)))

    y = np.concatenate([np.asarray(res.results[i]["y"]).reshape(-1)
                        for i in range(NCORES)])[:N]
    out = (y + np.float32(np.asarray(bl).reshape(-1)[0])).astype(np.float32)
    return out.reshape(N, 1)


# revision 10
# speedup vs baseline: 1.3221x; 1.3221x over previous
"""DCRNN (K=1, H0=0) fused kernel for 8 Trainium2 NeuronCores — fp8 v5.

Math (derived from the reference with H0 = 0):
    R is dead (multiplied by H0=0); XH == XHR == [x, 0].
    Az = (Wz[0] + Wz[1])[:F]           # [256, 32]
    Ah = (Wh[0] + Wh[1])[:F]           # [256, 32]
    Zc = sigmoid(-(x @ Az + bz))       # == 1 - Z, strictly positive
    T  = tanh(x @ Ah + bh) = 2*sigmoid(2*(x @ Ah + bh)) - 1
    y  = (relu(Zc * T) @ Wl) + bl
       = ((2*relu(T' - 0.5) * Zc) @ Wl) + bl,  T' = sigmoid(2*(x@Ah))

v5 strategy (per core, data-parallel over nodes):
  * x quantized host-side to fp8 e3m4 (xq = e3m4(c*x)); scale/sign/2x all
    folded into bf16 weights acat' = [-Az/c | 2*Ah/c].  HBM traffic
    halves vs bf16; whole 6.4MB shard preloaded to SBUF via large DMAs.
  * Node-stationary matmuls: lhsT = fp8 x-chunk [128,128] (FWL weight
    load), rhs = bf16 acat chunk [128,64]; psum groups of 3 superblocks.
  * ONE Sigmoid ACT per group covers both gates (contiguous psum read);
    a permuted 4D out-AP deposits z-cols into sg[:, :nz] and t-cols into
    sg[:, nz:2nz], both contiguous, so DVE runs at 2x/4x.
  * DVE: r = (T' max 0.5) + (-0.5) [one 4x tensor_scalar = relu(T'-1/2)],
    u = r*Zc (2x), g = u*(2*Wl) (2x).
  * GpSimd (otherwise idle): one tensor_reduce per group -> ysb columns.
  * y is PE-transposed in three flushes, stored via the sync HWDGE queue.
  * bz/bh are zero here; a generic variant adds them with a rank-1 matmul.
"""

import sys

import numpy as np

sys.path.insert(0, "/opt/trn_rl_repo")

import ml_dtypes

N = 200000
F = 256
HID = 32
NCORES = 8
PER = 25088            # padded nodes per core
NPAD = PER * NCORES    # 200704
SUPER = 1024           # nodes per superblock (8 subtiles of 128)
NSUPER = 25            # 24 full superblocks + 1 half (512 nodes)
YCOLS = PER // 128     # 196

XGROUPS = [1, 2, 4, 6, 6, 6]                    # x DMA groups (superblocks)
ACTG = [tuple(range(3 * i, 3 * i + 3)) for i in range(8)] + [(24,)]
_FLUSH = [(0, 96), (96, 192), (192, 196)]

BF16 = ml_dtypes.bfloat16
E3M4 = ml_dtypes.float8_e3m4
E3MAX = 15.5

_PROGS = {}


def _build_program(with_bias=False):
    import concourse.tile as tile
    from concourse import bacc, mybir

    BF = mybir.dt.bfloat16
    F8 = mybir.dt.float8e3
    F32 = mybir.dt.float32
    AF = mybir.ActivationFunctionType
    OP = mybir.AluOpType

    nc = bacc.Bacc("TRN2", target_bir_lowering=False, debug=False,
                   num_devices=NCORES)

    # host layout: [128, 50176] fp8; col = (b*2048 + c*1024 + j) for b<24,
    # tail b=24: 49152 + c*512 + j
    x_d = nc.dram_tensor("x", [128, 2 * PER], F8, kind="ExternalInput").ap()
    acat_d = nc.dram_tensor("acat", [2, 128, 64], BF, kind="ExternalInput").ap()
    wl_d = nc.dram_tensor("wl2full", [128, 768], BF, kind="ExternalInput").ap()
    id_d = nc.dram_tensor("ident", [128, 128], F32, kind="ExternalInput").ap()
    bias_d = nc.dram_tensor("biascat", [1, 512], BF, kind="ExternalInput").ap()
    ones_d = nc.dram_tensor("ones", [1, 128], BF, kind="ExternalInput").ap()
    y_d = nc.dram_tensor("y", [YCOLS, 128], F32, kind="ExternalOutput").ap()

    with tile.TileContext(nc) as tc:
        with tc.tile_pool(name="const", bufs=1) as cp, \
             tc.tile_pool(name="xs", bufs=len(XGROUPS)) as xp, \
             tc.tile_pool(name="sg", bufs=2) as gp, \
             tc.tile_pool(name="dv", bufs=8) as vp, \
             tc.tile_pool(name="ps", bufs=2, space="PSUM") as pp, \
             tc.tile_pool(name="yps", bufs=2, space="PSUM") as yp:

            acat0 = cp.tile([128, 64], BF)
            acat1 = cp.tile([128, 64], BF)
            wl2full = cp.tile([128, 768], BF)
            ident = cp.tile([128, 128], F32)
            ysb = cp.tile([128, YCOLS], F32)
            nc.scalar.dma_start(out=acat0[:], in_=acat_d[0])
            nc.scalar.dma_start(out=acat1[:], in_=acat_d[1])
            nc.scalar.dma_start(out=wl2full[:], in_=wl_d[:])
            nc.scalar.dma_start(out=ident[:], in_=id_d[:])
            if with_bias:
                biascat = cp.tile([1, 512], BF)
                ones = cp.tile([1, 128], BF)
                nc.scalar.dma_start(out=biascat[:], in_=bias_d[:])
                nc.scalar.dma_start(out=ones[:], in_=ones_d[:])

            # ---- preload the whole x shard (ramped DMAs, one HWDGE queue)
            xtiles = []      # (tile, first_super, n_supers)
            b0 = 0
            for ng in XGROUPS:
                xt = xp.tile([128, 12288], F8, tag="xt")
                c0 = b0 * 2048
                w = sum(2048 if (b0 + i) < NSUPER - 1 else 1024
                        for i in range(ng))
                nc.sync.dma_start(out=xt[:, :w], in_=x_d[:, c0:c0 + w])
                xtiles.append((xt, b0, ng))
                b0 += ng

            def lhs(b, c, s):
                """stationary fp8 x chunk [128, 128] for (superblock b,
                feature-chunk c, subtile s)."""
                for xt, g0, ng in xtiles:
                    if g0 <= b < g0 + ng:
                        nn = 1024 if b < NSUPER - 1 else 512
                        off = (b - g0) * 2048 + c * nn + s * 128
                        return xt[:, off:off + 128]
                raise AssertionError(b)

            # ---- main loop over ACT groups (3 superblocks each)
            ydone = 0
            yflush = 0
            for gi, supers in enumerate(ACTG):
                nsubg = sum((8 if b < NSUPER - 1 else 4) for b in supers)
                ncols = nsubg * 64
                nz = nsubg * 32
                pt = pp.tile([128, 1536], F32, tag="pt")
                col = 0
                for b in supers:
                    nsub = 8 if b < NSUPER - 1 else 4
                    for s in range(nsub):
                        out_sl = pt[:, col:col + 64]
                        if with_bias:
                            nc.tensor.matmul(out_sl, ones[:],
                                             biascat[:, :64],
                                             start=True, stop=False)
                        nc.tensor.matmul(out_sl, lhs(b, 0, s), acat0[:],
                                         start=not with_bias, stop=False)
                        nc.tensor.matmul(out_sl, lhs(b, 1, s), acat1[:],
                                         start=False, stop=True)
                        col += 64

                # ONE sigmoid for both gates: contiguous psum read, permuted
                # write -> z cols at sg[:, :nz], t' cols at sg[:, nz:2nz]
                sg = gp.tile([128, 1536], BF, tag="sg")
                in4 = pt[:, :ncols].rearrange("p (s c h) -> p s c h",
                                              c=2, h=32)
                out4 = sg[:, :2 * nz].rearrange("p (c s h) -> p s c h",
                                                c=2, h=32)
                nc.scalar.activation(out4, in4, AF.Sigmoid)
                sgz = sg[:, :nz]
                sgt = sg[:, nz:2 * nz]

                # DVE: r = relu(T' - 0.5) in ONE 4x tensor_scalar,
                # u = r*Zc (2x), g = u*(2*Wl) (2x)
                r = vp.tile([128, 768], BF, tag="r")
                nc.vector.tensor_scalar(r[:, :nz], sgt, 0.5, -0.5,
                                        op0=OP.max, op1=OP.add)
                u = vp.tile([128, 768], BF, tag="u")
                nc.vector.tensor_mul(u[:, :nz], r[:, :nz], sgz)
                g = vp.tile([128, 768], BF, tag="g")
                nc.vector.tensor_mul(g[:, :nz], u[:, :nz], wl2full[:, :nz])

                # reduce over HID: two add levels on GpSimd (idle engine),
                # final FD reduce on DVE
                g3 = g[:, :nz].rearrange("p (s h) -> p s h", h=32)
                t1 = vp.tile([128, 384], BF, tag="t1")
                t13 = t1[:, :16 * nsubg].rearrange("p (s h) -> p s h", h=16)
                nc.gpsimd.tensor_add(t13, g3[:, :, 0:16], g3[:, :, 16:32])
                t2 = vp.tile([128, 192], BF, tag="t2")
                t23 = t2[:, :8 * nsubg].rearrange("p (s h) -> p s h", h=8)
                nc.gpsimd.tensor_add(t23, t13[:, :, 0:8], t13[:, :, 8:16])
                yc0 = supers[0] * 8
                nc.vector.tensor_reduce(ysb[:, yc0:yc0 + nsubg], t23,
                                        axis=mybir.AxisListType.X, op=OP.add)
                ydone = yc0 + nsubg

                # flush finished slices of ysb to shorten the tail
                while yflush < len(_FLUSH) and ydone >= _FLUSH[yflush][1]:
                    h0, h1 = _FLUSH[yflush]
                    hw = h1 - h0
                    ytp = yp.tile([128, 128], F32, tag="ytp")
                    nc.tensor.transpose(ytp[:hw, :], ysb[:, h0:h1], ident[:])
                    yts = vp.tile([128, 128], F32, tag="yts")
                    nc.vector.tensor_copy(yts[:hw, :], ytp[:hw, :])
                    nc.sync.dma_start(out=y_d[h0:h1, :], in_=yts[:hw, :])
                    yflush += 1

    nc.compile()
    return nc


def _get_program(with_bias=False):
    if with_bias not in _PROGS:
        _PROGS[with_bias] = _build_program(with_bias)
    return _PROGS[with_bias]


def _host_inputs(x, Wz, bz, Wh, bh, Wl):
    x = np.asarray(x)
    Az = (np.asarray(Wz[0]) + np.asarray(Wz[1]))[:F]
    Ah = (np.asarray(Wh[0]) + np.asarray(Wh[1]))[:F]

    c = E3MAX / max(float(np.abs(x).max()), 1e-30)
    Acat = np.concatenate([-Az / c, (2.0 / c) * Ah], axis=1)  # [256, 64]
    acat = np.stack([Acat[:128], Acat[128:]]).astype(BF16)    # [2, 128, 64]
    wl2full = np.tile(np.asarray(Wl).reshape(1, HID) * 2.0,
                      (128, 24)).astype(BF16)                 # [128, 768]
    ident = np.eye(128, dtype=np.float32)
    biascat = np.concatenate([-np.asarray(bz), 2.0 * np.asarray(bh)])
    biascat8 = np.tile(biascat, 8)[None, :].astype(BF16)      # [1, 512]
    ones = np.ones((1, 128), BF16)

    # quantize + per-core transpose to [128, (b, c, j)] layout
    xq = np.clip(x * c, -E3MAX, E3MAX).astype(E3M4)
    xb = np.zeros((NPAD, F), dtype=E3M4)
    xb[:N] = xq
    shards = xb.reshape(NCORES, PER, F)
    nfull = (NSUPER - 1) * SUPER                              # 24576
    main = shards[:, :nfull].reshape(NCORES, NSUPER - 1, SUPER, 2, 128)
    main = main.transpose(0, 4, 1, 3, 2).reshape(NCORES, 128, -1)
    tail = shards[:, nfull:].reshape(NCORES, 1, PER - nfull, 2, 128)
    tail = tail.transpose(0, 4, 1, 3, 2).reshape(NCORES, 128, -1)
    xhost = np.concatenate([main, tail], axis=2)              # [NC, 128, 2*PER]
    return xhost, acat, wl2full, ident, biascat8, ones


def kernel(x, edge_index, Wz, bz, Wr, br, Wh, bh, Wl, bl):
    from concourse.bass_utils import run_bass_kernel_spmd

    xhost, acat, wl2full, ident, biascat8, ones = _host_inputs(
        x, Wz, bz, Wh, bh, Wl)
    with_bias = bool(np.any(np.asarray(bz)) or np.any(np.asarray(bh)))

    nc = _get_program(with_bias)
    in_maps = [{
        "x": np.ascontiguousarray(xhost[i]),
        "acat": acat,
        "wl2full": wl2full,
        "ident": ident,
        "biascat": biascat8,
        "ones": ones,
    } for i in range(NCORES)]

    res = run_bass_kernel_spmd(nc, in_maps, core_ids=list(range(NCORES)))

    y = np.concatenate([np.asarray(res.results[i]["y"]).reshape(-1)
                        for i in range(NCORES)])[:N]
    out = (y + np.float32(np.asarray(bl).reshape(-1)[0])).astype(np.float32)
    return out.reshape(N, 1)
